# revision 35
# baseline (speedup 1.0000x reference)
"""Trainium2 Bass kernel for the BDH-style sparse-attention model.

Model (per reference): L=6 layers over T=1024 tokens, D=256, H=4 heads,
N=32768 neurons (NH=8192 per head), strict-causal unnormalized linear
attention with RoPE over the neuron dim, gated wide projection, encoder
contraction with residual layernorms, final vocab readout.

Sharding (8 NeuronCores): tensor-parallel over (head, neuron): core c owns
head h=c//2 and half of that head's neurons (4096), chosen as a contiguous
slice of the rope *pair* space so rotary stays core-local.
Per layer:
  x   = relu(v @ Wx_c)            local [T, 4096]  (PE; Wx streamed)
  xr  = rope(x)                   local            (DVE+Pool, host tables)
  G   = xr xr^T strict-causal     fp8 DoubleRow PE (trapezoid blocks)
  a   = S^T-contract with v       partial -> pairwise AllReduce per head
  y   = relu(ln(a) @ Wy_c) * x    local (relu on Act, gate mult on DVE)
  e   = y @ Enc_c                 partial -> AllReduce over all 8 cores
  v   = ln(v + ln(e))             replicated
Output: v @ readout on every core (core 0's copy returned).

Engine placement: matmuls+transposes on PE (G in fp8e4 DoubleRow, rest
bf16), rope split DVE/GpSimd, relu/LN-apply/S-evac on Activation, gate
mult + stats on DVE. Collective staging DMAs ride the Act HWDGE ring so
they never queue behind the SP-ring weight streams.
"""

import numpy as np
import ml_dtypes

import concourse.bass as bass
import concourse.mybir as mybir
import concourse.tile as tile
from concourse import bacc
from concourse.bass_utils import run_bass_kernel_spmd

AF = mybir.ActivationFunctionType
ALU = mybir.AluOpType
F32 = mybir.dt.float32
BF16 = mybir.dt.bfloat16
FP8 = mybir.dt.float8e4

NCORES = 8
D = 256
H = 4
L = 6
N = 32768
NH = N // H          # 8192
NLOC = NH // 2       # 4096 per-core neurons
HALF = NLOC // 2     # 2048 rope pairs per core
T = 1024
VOCAB = 256
ROPE_BASE = 10000.0
NCH = NLOC // 128    # 32 chunks of 128 neurons
NPAIR = NCH // 2     # 16 rope chunk pairs
NDR = NCH // 2       # 16 fp8 DoubleRow chunk pairs
TCN = 2              # t-chunks
TCW = T // TCN       # 512
TT = T // 128        # 8 global t-tiles

REPLICA_PAIRS = [[0, 1], [2, 3], [4, 5], [6, 7]]
REPLICA_ALL = [list(range(NCORES))]


def build(nlayers: int = L, collectives: bool = True, fp8g: bool = True,
          pool_pairs: int = 0):
    nc = bacc.Bacc(
        "TRN2", target_bir_lowering=False, debug=False,
        enable_asserts=False, num_devices=NCORES,
    )

    # ---- DRAM I/O ----
    wx_d = nc.dram_tensor("wx", [NCH, 128, 2, 128], BF16, kind="ExternalInput")
    wy_d = nc.dram_tensor("wy", [NCH, 128, 2, 128], BF16, kind="ExternalInput")
    enc_d = nc.dram_tensor("enc", [NCH, 128, D], BF16, kind="ExternalInput")
    cs_d = nc.dram_tensor("cs", [128, NPAIR, T], BF16, kind="ExternalInput")
    sn_d = nc.dram_tensor("sn", [128, NPAIR, T], BF16, kind="ExternalInput")
    ro_d = nc.dram_tensor("ro", [128, 2, VOCAB], BF16, kind="ExternalInput")
    v0b_d = nc.dram_tensor("v0b", [128, TT, D], BF16, kind="ExternalInput")
    v0t_d = nc.dram_tensor("v0t", [128, 2, T], BF16, kind="ExternalInput")
    mask_d = nc.dram_tensor("maskd", [128, 128], BF16, kind="ExternalInput")
    ident_d = nc.dram_tensor("ident", [128, 128], BF16, kind="ExternalInput")
    identf_d = nc.dram_tensor("identf", [128, 128], F32, kind="ExternalInput")
    out_d = nc.dram_tensor("out", [T, VOCAB], F32, kind="ExternalOutput")

    wx_ap, wy_ap = wx_d.ap(), wy_d.ap()
    cs_ap, sn_ap, enc_ap = cs_d.ap(), sn_d.ap(), enc_d.ap()
    xr_dt = FP8 if fp8g else BF16

    with tile.TileContext(nc) as tc:
        with (
            tc.tile_pool(name="pers", bufs=1) as pers,
            tc.tile_pool(name="chbf", bufs=2) as chbf,
            tc.tile_pool(name="wstr", bufs=4) as wstr,
            tc.tile_pool(name="trig", bufs=2) as trig,
            tc.tile_pool(name="encp", bufs=4) as encp,
            tc.tile_pool(name="s32", bufs=1) as s32,
            tc.tile_pool(name="sbf", bufs=2) as sbf,
            tc.tile_pool(name="stat", bufs=2) as statp,
            tc.tile_pool(name="pxy", bufs=2, space="PSUM") as pxy_pool,
            tc.tile_pool(name="pgpe", bufs=4, space="PSUM") as pgpe_pool,
            tc.tile_pool(name="pap", bufs=2, space="PSUM") as pap_pool,
            tc.tile_pool(name="dram", bufs=2, space="DRAM") as dram,
        ):
            # ---- persistent SBUF ----
            ro = pers.tile([128, 2, VOCAB], BF16, tag="ro")
            maskd = pers.tile([128, 128], BF16, tag="maskd")
            ident = pers.tile([128, 128], BF16, tag="ident")
            identf = pers.tile([128, 128], F32, tag="identf")
            xT = pers.tile([128, NCH, T], BF16, tag="xT")
            # xr chunks packed as [pair m][k] for fp8 DoubleRow contraction
            xrT = pers.tile([128, NDR, 2, T], xr_dt, tag="xrT")
            S = pers.tile([128, TT, T], BF16, tag="S")
            vb = pers.tile([128, TT, D], BF16, tag="vb")
            vt = pers.tile([128, 2, T], BF16, tag="vt")
            eps = pers.tile([128, 1], F32, tag="eps")
            nc.gpsimd.memset(eps[:], 1e-5)

            nc.sync.dma_start(ro[:], ro_d[:])
            nc.sync.dma_start(maskd[:], mask_d[:])
            nc.sync.dma_start(ident[:], ident_d[:])
            nc.sync.dma_start(identf[:], identf_d[:])
            nc.sync.dma_start(vb[:], v0b_d[:])
            nc.sync.dma_start(vt[:], v0t_d[:])

            def xr_ap(cc, sl):
                return xrT[:, cc // 2, cc % 2, sl]

            def ln_stats(src, nt, tag):
                """src [128, nt, D] -> (rstd, -mean) each [128, nt].

                Only the Sqrt touches the Act engine; the apply is two Act
                passes so no DVE step sits between sqrt and apply."""
                bns = statp.tile([128, nt, 6], F32, tag=f"bns{tag}", bufs=2)
                agg = statp.tile([128, nt, 2], F32, tag=f"agg{tag}", bufs=2)
                for i in range(nt):
                    nc.vector.bn_stats(bns[:, i, :], src[:, i, :])
                    nc.vector.bn_aggr(agg[:, i, :], bns[:, i, :])
                std = statp.tile([128, nt], F32, tag=f"std{tag}", bufs=2)
                rstd = statp.tile([128, nt], F32, tag=f"rstd{tag}", bufs=2)
                nm = statp.tile([128, nt], F32, tag=f"nm{tag}", bufs=2)
                nc.scalar.activation(std[:], agg[:, :, 1], AF.Sqrt, bias=eps[:])
                nc.vector.reciprocal(rstd[:], std[:])
                nc.vector.scalar_tensor_tensor(
                    nm[:], agg[:, :, 0], -1.0, rstd[:], ALU.mult, ALU.mult)
                return rstd, nm

            def ln_apply(dst_ap_fn, src, nt, rstd, nm, mid_ap_fn=None):
                """dst = src*rstd + (-mean*rstd), one DVE op per tile —
                keeps the whole post-sqrt LN chain on the DVE."""
                for i in range(nt):
                    nc.vector.tensor_scalar(
                        dst_ap_fn(i), src[:, i, :], rstd[:, i:i + 1],
                        nm[:, i:i + 1], ALU.mult, ALU.add)

            def transpose_block(src_ap, dst_ap, name, dt=BF16):
                ps = pap_pool.tile([128, 128], dt, tag="pap", name=name)
                ident_ap = ident[:] if dt == BF16 else identf[:]
                nc.tensor.transpose(ps[:], src_ap, ident_ap)
                nc.any.tensor_copy(dst_ap, ps[:])

            a_red = {}
            e_red = {}

            def emit_xmm(l, tci):
                """x = relu(v @ Wx) for one t-chunk."""
                t0c = tci * TCW
                sl = slice(t0c, t0c + TCW)
                for cc in range(NCH):
                    wxs = wstr.tile([128, 2, 128], BF16, tag="wxs",
                                    bufs=6, name=f"wxs{cc}")
                    nc.sync.dma_start(wxs[:], wx_ap[cc])
                    ps = pxy_pool.tile([128, TCW], F32, tag="pxy",
                                       name=f"px_{cc}_{tci}")
                    for dc in range(2):
                        nc.tensor.matmul(
                            ps[:], wxs[:, dc, :],
                            vt[:, dc, sl],
                            start=(dc == 0), stop=(dc == 1),
                        )
                    nc.scalar.activation(xT[:, cc, sl], ps[:], AF.Relu)

            def emit_rope(l, tci):
                """xr = rope(x) for one t-chunk, two chunk-pairs per op.

                Pair-group g covers chunks (2g, 2g+1) == DoubleRow slot
                xrT[:, g, :, :], and (NPAIR+2g, NPAIR+2g+1) == slot
                xrT[:, NDR//2+g, :, :]."""
                t0c = tci * TCW
                sl = slice(t0c, t0c + TCW)
                for g in range(0, NPAIR // 2, 2):
                    co = trig.tile([128, 4, TCW], BF16, tag="cos",
                                   name=f"co_{g}")
                    si = trig.tile([128, 4, TCW], BF16, tag="sin",
                                   name=f"si_{g}")
                    nc.sync.dma_start(co[:], cs_ap[:, 2 * g:2 * g + 4, sl])
                    nc.sync.dma_start(si[:], sn_ap[:, 2 * g:2 * g + 4, sl])
                    x1 = xT[:, 2 * g:2 * g + 4, sl]
                    x2 = xT[:, NPAIR + 2 * g:NPAIR + 2 * g + 4, sl]
                    p1 = chbf.tile([128, 4, TCW], BF16, tag="p1", bufs=1,
                                   name=f"p1_{g}")
                    p2 = chbf.tile([128, 4, TCW], BF16, tag="p2", bufs=1,
                                   name=f"p2_{g}")
                    eng = nc.vector
                    # xr1 = x1*cos - x2*sin  (chunks 2g..2g+3 == slots g,g+1)
                    eng.tensor_tensor(p1[:], x1, co[:], ALU.mult)
                    eng.tensor_tensor(p2[:], x2, si[:], ALU.mult)
                    eng.tensor_tensor(
                        xrT[:, g:g + 2, :, sl], p1[:], p2[:], ALU.subtract)
                    # xr2 = x2*cos + x1*sin  (slots NDR//2+g, NDR//2+g+1)
                    eng.tensor_tensor(p1[:], x2, co[:], ALU.mult)
                    eng.tensor_tensor(p2[:], x1, si[:], ALU.mult)
                    eng.tensor_tensor(
                        xrT[:, NDR // 2 + g:NDR // 2 + g + 2, :, sl],
                        p1[:], p2[:], ALU.add)

            def emit_gsa(l, tci):
                """G blocks, S evac, a partial, AR(a) for one t-chunk."""
                t0c = tci * TCW
                # rope pair c covers chunk-pairs m=c//2 (xr1) and
                # NDR//2 + c//2 (xr2); order m so Pool pairs come last
                sts = list(range(4 * tci + 4))
                for g0 in range(0, len(sts), 4):
                    grp = sts[g0:g0 + 4]
                    pgs = {}
                    geom = {}
                    for st in grp:
                        tg0 = max(st * 128, t0c)
                        nw = t0c + TCW - tg0
                        geom[st] = (tg0, nw)
                        pgs[st] = pgpe_pool.tile(
                            [128, TCW], F32, tag="pgpe", name=f"pg_{st}")
                    if fp8g:
                        morder = [m for c2 in range(0, NPAIR, 2)
                                  for m in (c2 // 2, NDR // 2 + c2 // 2)]
                        for mi, m in enumerate(morder):
                            for st in grp:
                                tg0, nw = geom[st]
                                nc.tensor.matmul(
                                    pgs[st][:, :nw],
                                    xrT[:, m, :, st * 128:(st + 1) * 128],
                                    xrT[:, m, :, tg0:tg0 + nw],
                                    start=(mi == 0), stop=(mi == NDR - 1),
                                    perf_mode=mybir.MatmulPerfMode.DoubleRow,
                                )
                    else:
                        for cc in range(NCH):
                            for st in grp:
                                tg0, nw = geom[st]
                                nc.tensor.matmul(
                                    pgs[st][:, :nw],
                                    xr_ap(cc, slice(st * 128, (st + 1) * 128)),
                                    xr_ap(cc, slice(tg0, tg0 + nw)),
                                    start=(cc == 0), stop=(cc == NCH - 1),
                                )
                    for st in grp:
                        tg0, nw = geom[st]
                        pg = pgs[st]
                        if tg0 == st * 128:
                            nc.vector.tensor_tensor(
                                S[:, st, tg0:tg0 + 128], pg[:, 0:128], maskd[:],
                                ALU.mult)
                            if nw > 128:
                                nc.scalar.activation(
                                    S[:, st, tg0 + 128:tg0 + nw],
                                    pg[:, 128:nw], AF.Identity)
                        else:
                            nc.scalar.activation(
                                S[:, st, tg0:tg0 + nw], pg[:, :nw],
                                AF.Identity)

                # a partial
                a_loc = sbf.tile([128, 4, D], BF16, tag="a_loc", bufs=1,
                                 name=f"a_loc_{l}_{tci}")
                for i in range(4):
                    gt = 4 * tci + i
                    pa = pap_pool.tile([128, D], F32, tag="pap", name=f"pa_{gt}")
                    for st in range(gt + 1):
                        nc.tensor.matmul(
                            pa[:], S[:, st, gt * 128:(gt + 1) * 128],
                            vb[:, st, :],
                            start=(st == 0), stop=(st == gt),
                        )
                    nc.scalar.activation(a_loc[:, i, :], pa[:], AF.Identity)

                if collectives:
                    ain = dram.tile([128, 4, D], BF16, tag="ain",
                                    name=f"ain_{l}_{tci}")
                    aout = dram.tile([128, 4, D], BF16, tag="aout",
                                     name=f"aout_{l}_{tci}")
                    nc.scalar.dma_start(ain[:], a_loc[:])
                    if collectives == "dma":
                        nc.scalar.dma_start(aout[:], ain[:])
                    else:
                        nc.gpsimd.collective_compute(
                            "AllReduce", ALU.add, replica_groups=REPLICA_PAIRS,
                            ins=[ain.opt()], outs=[aout.opt()])
                    ar = sbf.tile([128, 4, D], BF16, tag="a_red", bufs=2,
                                  name=f"a_red_{l}_{tci}")
                    nc.scalar.dma_start(ar[:], aout[:])
                    a_red[tci] = ar
                else:
                    a_red[tci] = a_loc

            def emit_p2(l, tci):
                """ln(a); Y + gate + E; AR(e) for one t-chunk."""
                t0c = tci * TCW
                sl = slice(t0c, t0c + TCW)
                ar = a_red[tci]
                rstd, nmr = ln_stats(ar, 4, "a")
                lnA = sbf.tile([128, 4, D], BF16, tag="lnA", bufs=1,
                               name=f"lnA_{l}_{tci}")
                ln_apply(lambda i: lnA[:, i, :], ar, 4, rstd, nmr)
                lat = sbf.tile([128, 2, TCW], BF16, tag="lnAT", bufs=2,
                               name=f"lnAT_{l}_{tci}")
                for i in range(4):
                    for dc in range(2):
                        transpose_block(
                            lnA[:, i, dc * 128:(dc + 1) * 128],
                            lat[:, dc, i * 128:(i + 1) * 128],
                            f"ptA_{i}_{dc}")

                pe0 = pgpe_pool.tile([128, TCW], F32, tag="pgpe", name=f"pe0_{tci}")
                pe1 = pgpe_pool.tile([128, TCW], F32, tag="pgpe", name=f"pe1_{tci}")
                for c0 in range(0, NCH, 2):
                    yr = chbf.tile([128, 2, TCW], BF16, tag="yr",
                                   name=f"yr_{c0}")
                    for j in range(2):
                        c = c0 + j
                        wys = wstr.tile([128, 2, 128], BF16, tag="wys",
                                        bufs=6, name=f"wys{c}")
                        nc.sync.dma_start(wys[:], wy_ap[c])
                        py = pxy_pool.tile([128, TCW], F32, tag="pxy",
                                           name=f"py_{c}_{tci}")
                        for dc in range(2):
                            nc.tensor.matmul(
                                py[:], wys[:, dc, :], lat[:, dc, :],
                                start=(dc == 0), stop=(dc == 1),
                            )
                        nc.scalar.activation(yr[:, j, :], py[:], AF.Relu)
                    yc = chbf.tile([128, 2, TCW], BF16, tag="yc",
                                   name=f"yc_{c0}")
                    nc.vector.tensor_tensor(
                        yc[:], yr[:], xT[:, c0:c0 + 2, sl], ALU.mult)
                    for j in range(2):
                        c = c0 + j
                        ec = encp.tile([128, D], BF16, tag="enc", name=f"ec_{c}")
                        nc.sync.dma_start(ec[:], enc_ap[c, :, :])
                        for dc, pe in ((0, pe0), (1, pe1)):
                            nc.tensor.matmul(
                                pe[:], ec[:, dc * 128:(dc + 1) * 128],
                                yc[:, j, :],
                                start=(c == 0), stop=(c == NCH - 1),
                            )
                # evac e^T (f32), AllReduce in that layout, cast + transpose
                eT = sbf.tile([128, 2, TCW], BF16, tag="eT", bufs=2,
                              name=f"eT_{l}_{tci}")
                nc.vector.tensor_copy(eT[:, 0, :], pe0[:])
                nc.vector.tensor_copy(eT[:, 1, :], pe1[:])
                if collectives:
                    ein = dram.tile([128, 2, TCW], BF16, tag="ein",
                                    name=f"ein_{l}_{tci}")
                    eout = dram.tile([128, 2, TCW], BF16, tag="eout",
                                     name=f"eout_{l}_{tci}")
                    nc.scalar.dma_start(ein[:], eT[:])
                    if collectives == "dma":
                        nc.scalar.dma_start(eout[:], ein[:])
                    else:
                        nc.gpsimd.collective_compute(
                            "AllReduce", ALU.add, replica_groups=REPLICA_ALL,
                            ins=[ein.opt()], outs=[eout.opt()])
                    ert = sbf.tile([128, 2, TCW], BF16, tag="ert", bufs=2,
                                   name=f"ert_{l}_{tci}")
                    nc.scalar.dma_start(ert[:], eout[:])
                else:
                    ert = eT
                e_red[tci] = ert

            def emit_p3(l, tci):
                """transpose e, ln(e), residual, ln, v update (one t-chunk).

                Per-tile chains so the first vt transposes start after one
                tile's LN latency, not all four."""
                ert = e_red[tci]
                er = sbf.tile([128, 4, D], BF16, tag="e_red", bufs=2,
                              name=f"e_red_{l}_{tci}")
                lnE = s32.tile([128, 4, D], F32, tag="lnE", bufs=1,
                               name=f"lnE_{l}_{tci}")
                for i in range(4):
                    for dc in range(2):
                        transpose_block(
                            ert[:, dc, i * 128:(i + 1) * 128],
                            er[:, i, dc * 128:(dc + 1) * 128],
                            f"ptE_{i}_{dc}")
                rstd, nm = ln_stats(er, 4, "e")
                ln_apply(lambda i: lnE[:, i, :], er, 4, rstd, nm)
                nc.vector.tensor_tensor(
                    lnE[:], vb[:, 4 * tci:4 * tci + 4, :], lnE[:], ALU.add)
                rstd2, nm2 = ln_stats(lnE, 4, "v")
                for i in range(4):
                    gt = 4 * tci + i
                    nc.vector.tensor_scalar(
                        vb[:, gt, :], lnE[:, i, :], rstd2[:, i:i + 1],
                        nm2[:, i:i + 1], ALU.mult, ALU.add)
                    for dc in range(2):
                        transpose_block(
                            vb[:, gt, dc * 128:(dc + 1) * 128],
                            vt[:, dc, gt * 128:(gt + 1) * 128],
                            f"ptV_{gt}_{dc}")

            ob = s32.tile([128, TT, VOCAB], F32, tag="ob", bufs=1, name="ob")

            def emit_readout(gts):
                for gt in gts:
                    ps = pap_pool.tile([128, VOCAB], F32, tag="pap",
                                       name=f"pro_{gt}")
                    for dc in range(2):
                        nc.tensor.matmul(
                            ps[:], vt[:, dc, gt * 128:(gt + 1) * 128],
                            ro[:, dc, :],
                            start=(dc == 0), stop=(dc == 1),
                        )
                    nc.any.tensor_copy(ob[:, gt, :], ps[:])
                    nc.sync.dma_start(out_d[gt * 128:(gt + 1) * 128, :],
                                      ob[:, gt, :])

            # Software-pipelined emission: next layer's X matmuls are
            # emitted inside this layer's v-update windows, rope of chunk 1
            # is emitted after G/S/a of chunk 0 so the S evacuations don't
            # queue behind it on the DVE, and rope of the next layer's
            # chunk 0 rides the layer boundary.
            emit_xmm(0, 0)
            emit_xmm(0, 1)
            emit_rope(0, 0)
            for l in range(nlayers):
                emit_gsa(l, 0)
                emit_rope(l, 1)
                emit_gsa(l, 1)
                emit_p2(l, 0)
                emit_p2(l, 1)
                emit_p3(l, 0)
                if l + 1 < nlayers:
                    emit_xmm(l + 1, 0)
                else:
                    emit_readout(range(4))
                emit_p3(l, 1)
                if l + 1 < nlayers:
                    emit_xmm(l + 1, 1)
                    emit_rope(l + 1, 0)
                else:
                    emit_readout(range(4, TT))

            # readout emitted inside the last layer (per v-update chunk)

    nc.compile()
    return nc


def prep_inputs(inputs):
    """Full inputs -> per-core in_maps (host-side shard + precompute)."""
    bf = ml_dtypes.bfloat16
    idx = np.asarray(inputs["idx"], dtype=np.int32)
    wte = np.asarray(inputs["wte"], dtype=np.float32)
    enc = np.asarray(inputs["encoder"], dtype=np.float32)
    dx = np.asarray(inputs["decoder_x"], dtype=np.float32)
    dy = np.asarray(inputs["decoder_y"], dtype=np.float32)
    ro = np.asarray(inputs["readout"], dtype=np.float32)

    # embedding + initial layernorm (host)
    v0 = wte[idx[0]]
    m = v0.mean(-1, keepdims=True)
    va = v0.var(-1, keepdims=True)
    v0 = ((v0 - m) / np.sqrt(va + 1e-5)).astype(np.float32)  # [T, D]
    v0b = np.ascontiguousarray(
        v0.reshape(TT, 128, D).transpose(1, 0, 2)).astype(bf)
    v0t = np.ascontiguousarray(
        v0.T.reshape(2, 128, T).transpose(1, 0, 2)).astype(bf)

    half_g = NH // 2
    inv = 1.0 / (ROPE_BASE ** (np.arange(half_g, dtype=np.float32) / half_g))
    tarr = np.arange(T, dtype=np.float32)

    mask = np.triu(np.ones((128, 128), np.float32), k=1).astype(bf)
    ident = np.eye(128, dtype=np.float32).astype(bf)
    ro_arr = np.ascontiguousarray(
        ro.reshape(2, 128, VOCAB).transpose(1, 0, 2)).astype(bf)

    in_maps = []
    for c in range(NCORES):
        h, p = c // 2, c % 2
        j0, j1 = p * HALF, (p + 1) * HALF
        cols = np.r_[j0:j1, half_g + j0:half_g + j1]
        wx_c = dx[h][:, cols]   # [256, 4096]
        wy_c = dy[h][:, cols]
        enc_c = enc[h * NH:(h + 1) * NH][cols]  # [4096, 256]

        # [256, 4096] -> [NCH, 128, 2, 128]: [d, n] with d=128*dc+p, n=128*ch+i
        wx_arr = np.ascontiguousarray(
            wx_c.reshape(2, 128, NCH, 128).transpose(2, 1, 0, 3)).astype(bf)
        wy_arr = np.ascontiguousarray(
            wy_c.reshape(2, 128, NCH, 128).transpose(2, 1, 0, 3)).astype(bf)
        enc_arr = np.ascontiguousarray(enc_c.reshape(NCH, 128, D)).astype(bf)

        ang = tarr[:, None] * inv[None, j0:j1]      # [T, 2048]
        cos = np.cos(ang).T.astype(np.float32)      # [2048, T]
        sin = np.sin(ang).T.astype(np.float32)
        cs_arr = np.ascontiguousarray(
            cos.reshape(NPAIR, 128, T).transpose(1, 0, 2)).astype(bf)
        sn_arr = np.ascontiguousarray(
            sin.reshape(NPAIR, 128, T).transpose(1, 0, 2)).astype(bf)

        in_maps.append({
            "wx": wx_arr, "wy": wy_arr, "enc": enc_arr,
            "cs": cs_arr, "sn": sn_arr, "ro": ro_arr,
            "v0b": v0b, "v0t": v0t, "maskd": mask, "ident": ident,
            "identf": np.eye(128, dtype=np.float32),
        })
    return in_maps


_NC_CACHE = {}


def get_nc(nlayers: int = L):
    if nlayers not in _NC_CACHE:
        _NC_CACHE[nlayers] = build(nlayers)
    return _NC_CACHE[nlayers]


def kernel(**inputs) -> np.ndarray:
    nc = get_nc()
    in_maps = prep_inputs(inputs)
    res = run_bass_kernel_spmd(nc, in_maps, core_ids=list(range(NCORES)))
    out = res.results[0]["out"].astype(np.float32)
    return out.reshape(1, T, VOCAB)
